# revision 15
# baseline (speedup 1.0000x reference)
# Trainium2 Bass kernel for nn_LiteMultiscaleAttention (8-core data-parallel over batch).
#
# Per core (one batch). The dw5x5 + grouped-pw aggregation is channel-local
# within each 128-channel tile, so the whole network runs as ONE fused loop
# over 12 channel tiles:
#   ct loop:  qkv(ct) = Wqkv_ct @ x          (4 k-tile matmuls x 8 chunks)
#             agg(ct) = per-tap block-diag matmuls over padded qkv(ct)
#             route/gather q,k,v rows; attention stage A (vk^T) and stage B
#             (vk @ q / den) run as generators op-pumped into the matmul
#             stream so the PE never idles and HAM stays warm.
#   tail:     proj + BN pipelined chunk-wise into the last head-group drain,
#             attn staged via DRAM.
import sys
import numpy as np

sys.path.insert(0, '/opt/trn_rl_repo')

import ml_dtypes
import concourse.bass as bass
import concourse.mybir as mybir
import concourse.tile as tile
from concourse import bacc
from concourse.bass_utils import run_bass_kernel_spmd
from concourse.masks import make_identity

BF16 = mybir.dt.bfloat16
F32 = mybir.dt.float32

B, CIN, H, W = 8, 512, 64, 64
S = H * W                 # 4096
C3 = 1536                 # qkv channels
NCT = 12                  # channel tiles of qkv/agg
NKT = 4                   # k-tiles of Cin
NNT = 8                   # 512-wide spatial chunks (8 image rows each)
EPS = 1e-15
BN_EPS = 1e-5

_CACHED = {}


def build_program():
    nc = bacc.Bacc('TRN2', target_bir_lowering=False, debug=False)

    # ---------------- DRAM I/O ----------------
    x_d = nc.dram_tensor('x16', [128, NKT, S], BF16, kind='ExternalInput')
    wq_d = nc.dram_tensor('wq', [128, NKT, C3], BF16, kind='ExternalInput')
    lt_d = nc.dram_tensor('lt', [128, NCT, 25, 128], BF16, kind='ExternalInput')
    wp_d = nc.dram_tensor('wp', [128, 8, 512], BF16, kind='ExternalInput')
    bnb_d = nc.dram_tensor('bnb', [128, 4], F32, kind='ExternalInput')
    obd_d = nc.dram_tensor('obd', [128, 64], BF16, kind='ExternalInput')
    y_d = nc.dram_tensor('y_b', [512, S], F32, kind='ExternalOutput')
    # DRAM scratch: q rows (padded-rows layout for qkv heads, flat for agg heads)
    qstp_d = nc.dram_tensor('q_stp', [128, 4, 64, 68], BF16)
    qstf_d = nc.dram_tensor('q_stf', [128, 4, S], BF16)
    attn_d = nc.dram_tensor('attn_st', [128, 8, S], BF16)

    with tile.TileContext(nc) as tc:
        from contextlib import ExitStack
        ctx = ExitStack()
        with ctx:
            stat = ctx.enter_context(tc.tile_pool(name='stat', bufs=1))

            id128 = stat.tile([128, 128], BF16)
            make_identity(nc, id128[:])
            ones64 = stat.tile([128, 64], BF16)
            nc.sync.dma_start(ones64[:], obd_d.ap())

            # attention block-diag weights (built by stage A evacuations)
            bdn = stat.tile([128, 8, 64], BF16)
            nc.gpsimd.memset(bdn[:], 0.0)
            bdd = stat.tile([128, 8, 64], BF16)
            nc.gpsimd.memset(bdd[:], 0.0)
            den_col = stat.tile([128, 8, 1], F32)

            # persistent transpose staging tiles (vt has a ones column at 128)
            kt_t = [stat.tile([128, 128], BF16, name=f'ktt{i}') for i in range(2)]
            vt_t = [stat.tile([128, 132], BF16, name=f'vtt{i}') for i in range(2)]
            for i in range(2):
                nc.gpsimd.memset(vt_t[i][:, 128:129], 1.0)

            # rotating padded qkv channel-tile buffers; pad ring zeroed once
            qkv_bufs = [stat.tile([128, 68, 68], BF16, name=f'qkvb{i}')
                        for i in range(2)]
            for qb_ in qkv_bufs:
                nc.gpsimd.memset(qb_[:, 0:2, :], 0.0)
                nc.gpsimd.memset(qb_[:, 66:68, :], 0.0)
                nc.gpsimd.memset(qb_[:, 2:66, 0:2], 0.0)
                nc.gpsimd.memset(qb_[:, 2:66, 66:68], 0.0)

            # ---------------- PSUM pools (8 banks total) ----------------
            psum = ctx.enter_context(tc.tile_pool(name='psum', bufs=2, space='PSUM'))
            psA_pool = ctx.enter_context(tc.tile_pool(name='psA', bufs=1, space='PSUM'))
            trps = ctx.enter_context(tc.tile_pool(name='trps', bufs=2, space='PSUM'))
            psumB = ctx.enter_context(tc.tile_pool(name='psumB', bufs=2, space='PSUM'))

            # ---------------- SBUF pools (everything fits concurrently) -------
            es_p1 = ExitStack()     # x16: close before proj tiles alloc

            drp = ctx.enter_context(tc.tile_pool(name='drp', bufs=1))
            atcp = ctx.enter_context(tc.tile_pool(name='atcp', bufs=2))
            ltp = ctx.enter_context(tc.tile_pool(name='ltp', bufs=2))
            astp = ctx.enter_context(tc.tile_pool(name='astp', bufs=1))
            holdq = ctx.enter_context(tc.tile_pool(name='holdq', bufs=2))
            qbpp = ctx.enter_context(tc.tile_pool(name='qbpp', bufs=1))
            wqsp = ctx.enter_context(tc.tile_pool(name='wqsp', bufs=2))
            holda = ctx.enter_context(tc.tile_pool(name='holda', bufs=1))
            qbpf = ctx.enter_context(tc.tile_pool(name='qbpf', bufs=1))
            holdaL = ctx.enter_context(tc.tile_pool(name='holdaL', bufs=1))
            qbpfL = ctx.enter_context(tc.tile_pool(name='qbpfL', bufs=1))

            x16p = es_p1.enter_context(tc.tile_pool(name='x16p', bufs=1))
            x16 = x16p.tile([128, NKT, S], BF16)
            for kt in range(NKT):
                nc.sync.dma_start(x16[:, kt], x_d.ap()[:, kt])

            # ================= attention stage generators =================
            def stage_a_gen(tg, khold, vhold, padded):
                """Yield-granular stage A: per-head vk^T (+ k-sum via ones col).
                Padded holds are scanned as flat 128-wide windows; pad positions
                contribute relu(0)*0 = 0 to every accumulated product."""
                kfl = khold[:].rearrange('p a b -> p (a b)') if padded else khold[:]
                vfl = vhold[:].rearrange('p a b -> p (a b)') if padded else vhold[:]
                nst = 34 if padded else 32

                def win(t, st):
                    return t[:, 128 * st:128 * (st + 1)]
                nc.vector.tensor_scalar_max(khold[:], khold[:], 0.0)
                yield
                psa = psA_pool.tile([128, 132], F32, tag='psa')
                for st in range(nst):
                    kTt = kt_t[st % 2]
                    vTt = vt_t[st % 2]
                    psT = trps.tile([128, 128], BF16, tag='tr')
                    nc.tensor.transpose(psT[:], win(kfl, st), id128[:])
                    if st % 2 == 0:
                        nc.vector.tensor_copy(kTt[:], psT[:])
                    else:
                        nc.scalar.activation(kTt[:], psT[:],
                                             mybir.ActivationFunctionType.Copy)
                    yield
                    psT2 = trps.tile([128, 128], BF16, tag='tr')
                    nc.tensor.transpose(psT2[:], win(vfl, st), id128[:])
                    if st % 2 == 1:
                        nc.vector.tensor_copy(vTt[:, 0:128], psT2[:])
                    else:
                        nc.scalar.activation(vTt[:, 0:128], psT2[:],
                                             mybir.ActivationFunctionType.Copy)
                    yield
                    nc.tensor.matmul(psa[:, 0:129], kTt[:], vTt[:, 0:129],
                                     start=(st == 0), stop=(st == nst - 1))
                    yield
                # evacuate diag blocks; PSUM partition access must be 32-aligned,
                # so copy head-pairs [32,32] masked by the block-diag ones pattern
                for j in range(4):
                    r0 = 32 * j
                    cc = 32 * (j % 2)
                    nc.vector.scalar_tensor_tensor(
                        bdn[r0:r0 + 32, tg, cc:cc + 32],
                        psa[r0:r0 + 32, r0:r0 + 32], 1.0,
                        ones64[r0:r0 + 32, cc:cc + 32],
                        mybir.AluOpType.mult, mybir.AluOpType.mult)
                nc.vector.tensor_copy(den_col[:, tg, :], psa[:, 128:129])
                yield
                nc.vector.tensor_scalar_mul(bdd[0:64, tg, :], ones64[0:64, :],
                                            den_col[0:64, tg, :])
                nc.vector.tensor_scalar_mul(bdd[64:128, tg, :], ones64[64:128, :],
                                            den_col[64:128, tg, :])
                yield

            sb_chunk = {}

            def stage_b_gen(tg, padded, qpool=None):
                """Yield-granular stage B: attn = (vk @ relu(q)) / (ksum@q + eps)."""
                if padded:
                    qb = qbpp.tile([128, 64, 68], BF16, tag='qbp')
                    nc.sync.dma_start(qb[:], qstp_d.ap()[:, tg])
                else:
                    qb = qpool.tile([128, S], BF16, tag='qbf', name=f'qb{tg}')
                    nc.sync.dma_start(qb[:], qstf_d.ap()[:, tg - 4])
                yield
                nc.vector.tensor_scalar_max(qb[:], qb[:], 0.0)
                yield
                for nt in range(NNT):
                    if padded:
                        r = 8 * nt
                        qlo = qb[0:64, r:r + 8, 2:66]
                        qhi = qb[64:128, r:r + 8, 2:66]
                    else:
                        sl = slice(512 * nt, 512 * (nt + 1))
                        qlo = qb[0:64, sl]
                        qhi = qb[64:128, sl]
                    psN = psumB.tile([128, 512], F32, tag='psN', bufs=2)
                    nc.tensor.matmul(psN[0:64, :], bdn[0:64, tg, :], qlo,
                                     start=True, stop=True)
                    nc.tensor.matmul(psN[64:128, :], bdn[64:128, tg, :], qhi,
                                     start=True, stop=True)
                    yield
                    psD = psumB.tile([128, 512], F32, tag='psD', bufs=1)
                    nc.tensor.matmul(psD[0:64, :], bdd[0:64, tg, :], qlo,
                                     start=True, stop=True)
                    nc.tensor.matmul(psD[64:128, :], bdd[64:128, tg, :], qhi,
                                     start=True, stop=True)
                    yield
                    dre = drp.tile([128, 512], F32, tag='dre')
                    nc.scalar.activation(dre[:], psD[:],
                                         mybir.ActivationFunctionType.Copy, bias=EPS)
                    drt = drp.tile([128, 512], F32, tag='drt')
                    nc.vector.reciprocal_approx_fast(drt[:], dre[:])
                    yield
                    atc = atcp.tile([128, 512], BF16, tag='atc')
                    nc.vector.scalar_tensor_tensor(
                        atc[:], psN[:], 1.0, drt[:],
                        mybir.AluOpType.mult, mybir.AluOpType.mult)
                    nc.sync.dma_start(attn_d.ap()[:, tg, 512 * nt:512 * (nt + 1)],
                                      atc[:])
                    sb_chunk[tg] = nt
                    yield

            # generator pump: strict FIFO, one op-step per call
            pending = []

            def pump(n=1):
                for _ in range(n):
                    while pending:
                        try:
                            next(pending[0])
                            break
                        except StopIteration:
                            pending.pop(0)
                    else:
                        return

            # ---------------- fused main loop over channel tiles ----------------
            qkv_holdq = {}

            wqs_tiles = {0: wqsp.tile([128, NKT, 128], BF16, tag='wqs', name='wqs0')}
            nc.sync.dma_start(wqs_tiles[0][:], wq_d.ap()[:, :, 0:128])
            lt_tiles = {0: ltp.tile([128, 25, 128], BF16, tag='lt', name='lt0')}
            nc.sync.dma_start(lt_tiles[0][:], lt_d.ap()[:, 0])
            agg_khold = {}
            agg_vhold = {}
            for ct in range(NCT):
                if ct + 1 < NCT:
                    wqs_tiles[ct + 1] = wqsp.tile([128, NKT, 128], BF16, tag='wqs',
                                                  name=f'wqs{ct+1}')
                    nc.sync.dma_start(wqs_tiles[ct + 1][:],
                                      wq_d.ap()[:, :, 128 * (ct + 1):128 * (ct + 2)])
                    lt_tiles[ct + 1] = ltp.tile([128, 25, 128], BF16, tag='lt',
                                                name=f'lt{ct+1}')
                    nc.sync.dma_start(lt_tiles[ct + 1][:], lt_d.ap()[:, ct + 1])
                wqs = wqs_tiles.pop(ct)
                lt = lt_tiles.pop(ct)
                qkvb = qkv_bufs[ct % 2]

                # qkv 1x1 conv for this channel tile
                for nt in range(NNT):
                    ps = psum.tile([128, 512], F32, tag='ps512')
                    for kt in range(NKT):
                        nc.tensor.matmul(
                            ps[:], wqs[:, kt, :],
                            x16[:, kt, 512 * nt:512 * (nt + 1)],
                            start=(kt == 0), stop=(kt == NKT - 1))
                    dst = qkvb[:, 8 * nt + 2:8 * nt + 10, 2:66]
                    if nt % 2 == 0:
                        nc.vector.tensor_copy(dst, ps[:])
                    else:
                        nc.scalar.activation(dst, ps[:],
                                             mybir.ActivationFunctionType.Copy)
                    pump(2)

                # fused dw5x5 + grouped-pw taps (channel-local)
                ast = astp.tile([128, S], BF16, tag='ast', name=f'ast{ct}')
                for nt in range(NNT):
                    ps = psum.tile([128, 512], F32, tag='ps512')
                    for tap in range(25):
                        dy, dx = tap // 5, tap % 5
                        rhs = qkvb[:, 8 * nt + dy:8 * nt + dy + 8, dx:dx + 64]
                        nc.tensor.matmul(ps[:], lt[:, tap, :], rhs,
                                         start=(tap == 0), stop=(tap == 24))
                        if tap % 2 == 0:
                            pump(1)
                    dst = ast[:, 512 * nt:512 * (nt + 1)]
                    if nt % 2 == 0:
                        nc.vector.tensor_copy(dst, ps[:])
                    else:
                        nc.scalar.activation(dst, ps[:],
                                             mybir.ActivationFunctionType.Copy)

                # gather this tile's qkv-branch q/k/v rows (contiguous sources)
                tgq = ct // 3
                if ct % 3 == 0:
                    qkv_holdq[tgq] = (
                        holdq.tile([128, 64, 68], BF16, tag='kh', name=f'qkh{tgq}'),
                        holdq.tile([128, 64, 68], BF16, tag='vh', name=f'qvh{tgq}'))
                khq, vhq = qkv_holdq[tgq]
                for bi in range(8):
                    c = 128 * ct + 16 * bi
                    h = c // 48
                    r = c % 48
                    hl = h % 8
                    src = qkvb[16 * bi:16 * bi + 16, 2:66, :]
                    if r == 0:
                        nc.sync.dma_start(
                            qstp_d.ap()[16 * hl:16 * hl + 16, tgq], src)
                    elif r == 16:
                        nc.gpsimd.dma_start(khq[16 * hl:16 * hl + 16], src)
                    else:
                        nc.gpsimd.dma_start(vhq[16 * hl:16 * hl + 16], src)

                # route this tile's agg 16-row blocks to their destinations
                for bi in range(8):
                    c = 128 * ct + 16 * bi
                    h = 32 + c // 48
                    r = c % 48
                    tg = h // 8
                    hl = h % 8
                    if tg not in agg_khold:
                        hp = holdaL if tg == 7 else holda
                        agg_khold[tg] = hp.tile([128, S], BF16, tag='akh',
                                                name=f'akh{tg}')
                        agg_vhold[tg] = hp.tile([128, S], BF16, tag='avh',
                                                name=f'avh{tg}')
                    blk = ast[16 * bi:16 * bi + 16, :]
                    if r == 0:
                        nc.sync.dma_start(
                            qstf_d.ap()[16 * hl:16 * hl + 16, tg - 4], blk)
                    elif r == 16:
                        nc.sync.dma_start(agg_khold[tg][16 * hl:16 * hl + 16, :], blk)
                    else:
                        nc.sync.dma_start(agg_vhold[tg][16 * hl:16 * hl + 16, :], blk)

                # activate stage generators once a head group is complete
                if ct % 3 == 2:
                    kh, vh = qkv_holdq.pop(tgq)
                    pending.append(stage_a_gen(tgq, kh, vh, padded=True))
                    pending.append(stage_b_gen(tgq, padded=True))
                    tga = 4 + ct // 3
                    kh = agg_khold.pop(tga)
                    vh = agg_vhold.pop(tga)
                    qp = qbpfL if tga == 7 else qbpf
                    pending.append(stage_a_gen(tga, kh, vh, padded=False))
                    pending.append(stage_b_gen(tga, padded=False, qpool=qp))

            # ---------------- proj + BN, pipelined into the stage drain --------
            es_p1.close()
            wpp = ctx.enter_context(tc.tile_pool(name='wpp', bufs=1))
            wp = wpp.tile([128, 8, 512], BF16)
            nc.sync.dma_start(wp[:], wp_d.ap())
            bnbp = ctx.enter_context(tc.tile_pool(name='bnbp', bufs=1))
            bnb = bnbp.tile([128, 4], F32)
            nc.sync.dma_start(bnb[:], bnb_d.ap())

            atp = ctx.enter_context(tc.tile_pool(name='atp', bufs=2))
            ysp = ctx.enter_context(tc.tile_pool(name='ysp', bufs=3))
            at_tiles = {}

            def fetch_at(nt):
                # attn chunk nt is fully written once every tg emitted it
                while pending and min(sb_chunk.get(t, -1) for t in range(8)) < nt:
                    pump(1)
                at_tiles[nt] = atp.tile([128, 8, 512], BF16, tag='at',
                                        name=f'at{nt}')
                nc.sync.dma_start(at_tiles[nt][:],
                                  attn_d.ap()[:, :, 512 * nt:512 * (nt + 1)])

            fetch_at(0)
            for nt in range(NNT):
                if nt + 1 < NNT:
                    fetch_at(nt + 1)
                at = at_tiles.pop(nt)
                pump(2)
                for mt in range(4):
                    ps = psumB.tile([128, 512], F32, tag='psN', bufs=2)
                    for kt in range(8):
                        nc.tensor.matmul(ps[:], wp[:, kt, 128 * mt:128 * (mt + 1)],
                                         at[:, kt, :],
                                         start=(kt == 0), stop=(kt == 7))
                    ys = ysp.tile([128, 512], F32, tag='ys')
                    if mt % 2 == 0:
                        nc.vector.tensor_scalar_add(ys[:], ps[:], bnb[:, mt:mt + 1])
                    else:
                        nc.scalar.activation(ys[:], ps[:],
                                             mybir.ActivationFunctionType.Identity,
                                             bias=bnb[:, mt:mt + 1])
                    nc.sync.dma_start(
                        y_d.ap()[128 * mt:128 * (mt + 1), 512 * nt:512 * (nt + 1)],
                        ys[:])
            while pending:
                pump(1)

    nc.compile()
    return nc


def host_weights(w_qkv, w_dw, w_pw, w_proj, bn_gamma, bn_beta, bn_mean, bn_var):
    wq = w_qkv[:, :, 0, 0].astype(np.float32)       # [1536, 512]
    wdw = w_dw[:, 0].reshape(1536, 25).astype(np.float32)
    wpw = w_pw[:, :, 0, 0].astype(np.float32)       # [1536, 32]
    A = wdw.reshape(12, 4, 32, 25)
    Bm = wpw.reshape(12, 4, 32, 32)
    # W2c[ct, b, il, tap, ol] = dw[in-ch, tap] * pw[out-ch, in-ch]
    W2c = A[:, :, :, :, None] * Bm.transpose(0, 1, 3, 2)[:, :, :, None, :]
    lt = np.zeros((128, 12, 25, 128), np.float32)
    for b in range(4):
        lt[32 * b:32 * b + 32, :, :, 32 * b:32 * b + 32] = \
            W2c[:, b].transpose(1, 0, 2, 3)
    inv = bn_gamma / np.sqrt(bn_var + BN_EPS)
    wp_f = (w_proj[:, :, 0, 0] * inv[:, None]).T    # [1024, 512] lhsT
    bnb = (bn_beta - bn_mean * inv).astype(np.float32)

    wq_dev = np.ascontiguousarray(
        wq.T.reshape(NKT, 128, C3).transpose(1, 0, 2)).astype(ml_dtypes.bfloat16)
    lt_dev = lt.astype(ml_dtypes.bfloat16)
    wp_dev = np.ascontiguousarray(
        wp_f.reshape(8, 128, 512).transpose(1, 0, 2)).astype(ml_dtypes.bfloat16)
    bnb_dev = np.ascontiguousarray(bnb.reshape(4, 128).T).astype(np.float32)
    obd = np.zeros((128, 64), np.float32)
    for half in (0, 64):
        for j in range(4):
            obd[half + 16 * j:half + 16 * j + 16, 16 * j:16 * j + 16] = 1.0
    obd_dev = obd.astype(ml_dtypes.bfloat16)
    return {'wq': wq_dev, 'lt': lt_dev, 'wp': wp_dev, 'bnb': bnb_dev, 'obd': obd_dev}


def kernel(x, w_qkv, w_dw, w_pw, w_proj, bn_gamma, bn_beta, bn_mean, bn_var):
    x = np.asarray(x, dtype=np.float32)
    wdev = host_weights(
        np.asarray(w_qkv, np.float32), np.asarray(w_dw, np.float32),
        np.asarray(w_pw, np.float32), np.asarray(w_proj, np.float32),
        np.asarray(bn_gamma, np.float32), np.asarray(bn_beta, np.float32),
        np.asarray(bn_mean, np.float32), np.asarray(bn_var, np.float32))

    if 'nc' not in _CACHED:
        _CACHED['nc'] = build_program()
    nc = _CACHED['nc']

    in_maps = []
    for b in range(B):
        xb = np.ascontiguousarray(
            x[b].reshape(NKT, 128, S).transpose(1, 0, 2)).astype(ml_dtypes.bfloat16)
        in_maps.append({'x16': xb, **wdev})
    _CACHED['in_maps'] = in_maps
    res = run_bass_kernel_spmd(nc, in_maps, list(range(B)))
    y = np.stack([res.results[b]['y_b'].reshape(512, H, W) for b in range(B)])
    return y.astype(np.float32)


# revision 16
# speedup vs baseline: 1.0140x; 1.0140x over previous
# Trainium2 Bass kernel for nn_LiteMultiscaleAttention (8-core data-parallel over batch).
#
# Per core (one batch). The dw5x5 + grouped-pw aggregation is channel-local
# within each 128-channel tile, so the whole network runs as ONE fused loop
# over 12 channel tiles:
#   ct loop:  qkv(ct) = Wqkv_ct @ x          (4 k-tile matmuls x 8 chunks)
#             agg(ct) = per-tap block-diag matmuls over padded qkv(ct)
#             route/gather q,k,v rows; attention stage A (vk^T) and stage B
#             (vk @ q / den) run as generators op-pumped into the matmul
#             stream so the PE never idles and HAM stays warm.
#   tail:     proj + BN pipelined chunk-wise into the last head-group drain,
#             attn staged via DRAM.
import sys
import numpy as np

sys.path.insert(0, '/opt/trn_rl_repo')

import ml_dtypes
import concourse.bass as bass
import concourse.mybir as mybir
import concourse.tile as tile
from concourse import bacc
from concourse.bass_utils import run_bass_kernel_spmd
from concourse.masks import make_identity

BF16 = mybir.dt.bfloat16
F32 = mybir.dt.float32

B, CIN, H, W = 8, 512, 64, 64
S = H * W                 # 4096
C3 = 1536                 # qkv channels
NCT = 12                  # channel tiles of qkv/agg
NKT = 4                   # k-tiles of Cin
NNT = 8                   # 512-wide spatial chunks (8 image rows each)
EPS = 1e-15
BN_EPS = 1e-5

_CACHED = {}


def build_program():
    nc = bacc.Bacc('TRN2', target_bir_lowering=False, debug=False)

    # ---------------- DRAM I/O ----------------
    x_d = nc.dram_tensor('x16', [128, NKT, S], BF16, kind='ExternalInput')
    wq_d = nc.dram_tensor('wq', [128, NKT, C3], BF16, kind='ExternalInput')
    lt_d = nc.dram_tensor('lt', [128, NCT, 25, 128], BF16, kind='ExternalInput')
    wp_d = nc.dram_tensor('wp', [128, 8, 512], BF16, kind='ExternalInput')
    bnb_d = nc.dram_tensor('bnb', [128, 4], F32, kind='ExternalInput')
    obd_d = nc.dram_tensor('obd', [128, 128], BF16, kind='ExternalInput')
    y_d = nc.dram_tensor('y_b', [512, S], F32, kind='ExternalOutput')
    # DRAM scratch: q rows (padded-rows layout for qkv heads, flat for agg heads)
    qstp_d = nc.dram_tensor('q_stp', [128, 4, 64, 68], BF16)
    qstf_d = nc.dram_tensor('q_stf', [128, 4, S], BF16)
    attn_d = nc.dram_tensor('attn_st', [128, 8, S], BF16)

    with tile.TileContext(nc) as tc:
        from contextlib import ExitStack
        ctx = ExitStack()
        with ctx:
            stat = ctx.enter_context(tc.tile_pool(name='stat', bufs=1))

            id128 = stat.tile([128, 128], BF16)
            make_identity(nc, id128[:])
            onesF = stat.tile([128, 128], BF16)
            nc.sync.dma_start(onesF[:], obd_d.ap())

            # attention block-diag weights (built by stage A evacuations)
            bdn = stat.tile([128, 8, 128], BF16)
            nc.gpsimd.memset(bdn[:], 0.0)
            bdd = stat.tile([128, 8, 128], BF16)
            den_col = stat.tile([128, 8, 1], F32)

            # persistent transpose staging tiles (vt has a ones column at 128)
            kt_t = [stat.tile([128, 128], BF16, name=f'ktt{i}') for i in range(2)]
            vt_t = [stat.tile([128, 132], BF16, name=f'vtt{i}') for i in range(2)]
            for i in range(2):
                nc.gpsimd.memset(vt_t[i][:, 128:129], 1.0)

            # rotating padded qkv channel-tile buffers; pad ring zeroed once
            qkv_bufs = [stat.tile([128, 68, 68], BF16, name=f'qkvb{i}')
                        for i in range(2)]
            for qb_ in qkv_bufs:
                nc.gpsimd.memset(qb_[:, 0:2, :], 0.0)
                nc.gpsimd.memset(qb_[:, 66:68, :], 0.0)
                nc.gpsimd.memset(qb_[:, 2:66, 0:2], 0.0)
                nc.gpsimd.memset(qb_[:, 2:66, 66:68], 0.0)

            # ---------------- PSUM pools (8 banks total) ----------------
            psum = ctx.enter_context(tc.tile_pool(name='psum', bufs=2, space='PSUM'))
            psA_pool = ctx.enter_context(tc.tile_pool(name='psA', bufs=1, space='PSUM'))
            trps = ctx.enter_context(tc.tile_pool(name='trps', bufs=2, space='PSUM'))
            psumB = ctx.enter_context(tc.tile_pool(name='psumB', bufs=2, space='PSUM'))

            # ---------------- SBUF pools (everything fits concurrently) -------
            es_p1 = ExitStack()     # x16: close before proj tiles alloc

            drp = ctx.enter_context(tc.tile_pool(name='drp', bufs=1))
            atcp = ctx.enter_context(tc.tile_pool(name='atcp', bufs=2))
            ltp = ctx.enter_context(tc.tile_pool(name='ltp', bufs=2))
            astp = ctx.enter_context(tc.tile_pool(name='astp', bufs=1))
            holdq = ctx.enter_context(tc.tile_pool(name='holdq', bufs=2))
            qbpp = ctx.enter_context(tc.tile_pool(name='qbpp', bufs=1))
            wqsp = ctx.enter_context(tc.tile_pool(name='wqsp', bufs=2))
            holda = ctx.enter_context(tc.tile_pool(name='holda', bufs=1))
            qbpf = ctx.enter_context(tc.tile_pool(name='qbpf', bufs=1))
            holdaL = ctx.enter_context(tc.tile_pool(name='holdaL', bufs=1))
            qbpfL = ctx.enter_context(tc.tile_pool(name='qbpfL', bufs=1))

            x16p = es_p1.enter_context(tc.tile_pool(name='x16p', bufs=1))
            x16 = x16p.tile([128, NKT, S], BF16)
            for kt in range(NKT):
                nc.sync.dma_start(x16[:, kt], x_d.ap()[:, kt])

            # ================= attention stage generators =================
            def stage_a_gen(tg, khold, vhold, padded):
                """Yield-granular stage A: per-head vk^T (+ k-sum via ones col).
                Padded holds are scanned as flat 128-wide windows; pad positions
                contribute relu(0)*0 = 0 to every accumulated product."""
                kfl = khold[:].rearrange('p a b -> p (a b)') if padded else khold[:]
                vfl = vhold[:].rearrange('p a b -> p (a b)') if padded else vhold[:]
                nst = 34 if padded else 32

                def win(t, st):
                    return t[:, 128 * st:128 * (st + 1)]
                nc.vector.tensor_scalar_max(khold[:], khold[:], 0.0)
                yield
                psa = psA_pool.tile([128, 132], F32, tag='psa')
                for st in range(nst):
                    kTt = kt_t[st % 2]
                    vTt = vt_t[st % 2]
                    # transpose as a REGULAR matmul (chunk^T @ I): identical
                    # result, but counts as PE activity for the HAM clock
                    # gate and avoids transpose-mode switch drains
                    psT = trps.tile([128, 128], F32, tag='tr')
                    nc.tensor.matmul(psT[:], win(kfl, st), id128[:],
                                     start=True, stop=True)
                    if st % 2 == 0:
                        nc.vector.tensor_copy(kTt[:], psT[:])
                    else:
                        nc.scalar.activation(kTt[:], psT[:],
                                             mybir.ActivationFunctionType.Copy)
                    yield
                    psT2 = trps.tile([128, 128], F32, tag='tr')
                    nc.tensor.matmul(psT2[:], win(vfl, st), id128[:],
                                     start=True, stop=True)
                    if st % 2 == 1:
                        nc.vector.tensor_copy(vTt[:, 0:128], psT2[:])
                    else:
                        nc.scalar.activation(vTt[:, 0:128], psT2[:],
                                             mybir.ActivationFunctionType.Copy)
                    yield
                    nc.tensor.matmul(psa[:, 0:129], kTt[:], vTt[:, 0:129],
                                     start=(st == 0), stop=(st == nst - 1))
                    yield
                # evacuate diag blocks; PSUM partition access must be 32-aligned,
                # so copy head-pairs [32,32] masked by the block-diag ones pattern
                for j in range(4):
                    r0 = 32 * j
                    nc.vector.scalar_tensor_tensor(
                        bdn[r0:r0 + 32, tg, r0:r0 + 32],
                        psa[r0:r0 + 32, r0:r0 + 32], 1.0,
                        onesF[r0:r0 + 32, r0:r0 + 32],
                        mybir.AluOpType.mult, mybir.AluOpType.mult)
                nc.vector.tensor_copy(den_col[:, tg, :], psa[:, 128:129])
                yield
                nc.vector.tensor_scalar_mul(bdd[:, tg, :], onesF[:],
                                            den_col[:, tg, :])
                yield

            sb_chunk = {}

            def stage_b_gen(tg, padded, qpool=None):
                """Yield-granular stage B: attn = (vk @ relu(q)) / (ksum@q + eps)."""
                if padded:
                    qb = qbpp.tile([128, 64, 68], BF16, tag='qbp')
                    nc.sync.dma_start(qb[:], qstp_d.ap()[:, tg])
                else:
                    qb = qpool.tile([128, S], BF16, tag='qbf', name=f'qb{tg}')
                    nc.sync.dma_start(qb[:], qstf_d.ap()[:, tg - 4])
                yield
                nc.vector.tensor_scalar_max(qb[:], qb[:], 0.0)
                yield
                for nt in range(NNT):
                    if padded:
                        r = 8 * nt
                        qw = qb[:, r:r + 8, 2:66]
                    else:
                        qw = qb[:, 512 * nt:512 * (nt + 1)]
                    psN = psumB.tile([128, 512], F32, tag='psN', bufs=2)
                    nc.tensor.matmul(psN[:], bdn[:, tg, :], qw,
                                     start=True, stop=True)
                    yield
                    psD = psumB.tile([128, 512], F32, tag='psD', bufs=1)
                    nc.tensor.matmul(psD[:], bdd[:, tg, :], qw,
                                     start=True, stop=True)
                    yield
                    dre = drp.tile([128, 512], F32, tag='dre')
                    nc.scalar.activation(dre[:], psD[:],
                                         mybir.ActivationFunctionType.Copy, bias=EPS)
                    drt = drp.tile([128, 512], F32, tag='drt')
                    nc.vector.reciprocal_approx_fast(drt[:], dre[:])
                    yield
                    atc = atcp.tile([128, 512], BF16, tag='atc')
                    nc.vector.scalar_tensor_tensor(
                        atc[:], psN[:], 1.0, drt[:],
                        mybir.AluOpType.mult, mybir.AluOpType.mult)
                    nc.sync.dma_start(attn_d.ap()[:, tg, 512 * nt:512 * (nt + 1)],
                                      atc[:])
                    sb_chunk[tg] = nt
                    yield

            # generator pump: strict FIFO, one op-step per call
            pending = []

            def pump(n=1):
                for _ in range(n):
                    while pending:
                        try:
                            next(pending[0])
                            break
                        except StopIteration:
                            pending.pop(0)
                    else:
                        return

            # ---------------- fused main loop over channel tiles ----------------
            qkv_holdq = {}

            wqs_tiles = {0: wqsp.tile([128, NKT, 128], BF16, tag='wqs', name='wqs0')}
            nc.sync.dma_start(wqs_tiles[0][:], wq_d.ap()[:, :, 0:128])
            lt_tiles = {0: ltp.tile([128, 25, 128], BF16, tag='lt', name='lt0')}
            nc.sync.dma_start(lt_tiles[0][:], lt_d.ap()[:, 0])
            agg_khold = {}
            agg_vhold = {}
            for ct in range(NCT):
                if ct + 1 < NCT:
                    wqs_tiles[ct + 1] = wqsp.tile([128, NKT, 128], BF16, tag='wqs',
                                                  name=f'wqs{ct+1}')
                    nc.sync.dma_start(wqs_tiles[ct + 1][:],
                                      wq_d.ap()[:, :, 128 * (ct + 1):128 * (ct + 2)])
                    lt_tiles[ct + 1] = ltp.tile([128, 25, 128], BF16, tag='lt',
                                                name=f'lt{ct+1}')
                    nc.sync.dma_start(lt_tiles[ct + 1][:], lt_d.ap()[:, ct + 1])
                wqs = wqs_tiles.pop(ct)
                lt = lt_tiles.pop(ct)
                qkvb = qkv_bufs[ct % 2]

                # qkv 1x1 conv for this channel tile
                for nt in range(NNT):
                    ps = psum.tile([128, 512], F32, tag='ps512')
                    for kt in range(NKT):
                        nc.tensor.matmul(
                            ps[:], wqs[:, kt, :],
                            x16[:, kt, 512 * nt:512 * (nt + 1)],
                            start=(kt == 0), stop=(kt == NKT - 1))
                    dst = qkvb[:, 8 * nt + 2:8 * nt + 10, 2:66]
                    if nt % 2 == 0:
                        nc.vector.tensor_copy(dst, ps[:])
                    else:
                        nc.scalar.activation(dst, ps[:],
                                             mybir.ActivationFunctionType.Copy)
                    pump(2)

                # fused dw5x5 + grouped-pw taps (channel-local)
                ast = astp.tile([128, S], BF16, tag='ast', name=f'ast{ct}')
                for nt in range(NNT):
                    ps = psum.tile([128, 512], F32, tag='ps512')
                    for tap in range(25):
                        dy, dx = tap // 5, tap % 5
                        rhs = qkvb[:, 8 * nt + dy:8 * nt + dy + 8, dx:dx + 64]
                        nc.tensor.matmul(ps[:], lt[:, tap, :], rhs,
                                         start=(tap == 0), stop=(tap == 24))
                        if tap % 2 == 0:
                            pump(1)
                    dst = ast[:, 512 * nt:512 * (nt + 1)]
                    if nt % 2 == 0:
                        nc.vector.tensor_copy(dst, ps[:])
                    else:
                        nc.scalar.activation(dst, ps[:],
                                             mybir.ActivationFunctionType.Copy)

                # gather this tile's qkv-branch q/k/v rows (contiguous sources)
                tgq = ct // 3
                if ct % 3 == 0:
                    qkv_holdq[tgq] = (
                        holdq.tile([128, 64, 68], BF16, tag='kh', name=f'qkh{tgq}'),
                        holdq.tile([128, 64, 68], BF16, tag='vh', name=f'qvh{tgq}'))
                khq, vhq = qkv_holdq[tgq]
                for bi in range(8):
                    c = 128 * ct + 16 * bi
                    h = c // 48
                    r = c % 48
                    hl = h % 8
                    src = qkvb[16 * bi:16 * bi + 16, 2:66, :]
                    if r == 0:
                        nc.sync.dma_start(
                            qstp_d.ap()[16 * hl:16 * hl + 16, tgq], src)
                    elif r == 16:
                        nc.gpsimd.dma_start(khq[16 * hl:16 * hl + 16], src)
                    else:
                        nc.gpsimd.dma_start(vhq[16 * hl:16 * hl + 16], src)

                # route this tile's agg 16-row blocks to their destinations
                for bi in range(8):
                    c = 128 * ct + 16 * bi
                    h = 32 + c // 48
                    r = c % 48
                    tg = h // 8
                    hl = h % 8
                    if tg not in agg_khold:
                        hp = holdaL if tg == 7 else holda
                        agg_khold[tg] = hp.tile([128, S], BF16, tag='akh',
                                                name=f'akh{tg}')
                        agg_vhold[tg] = hp.tile([128, S], BF16, tag='avh',
                                                name=f'avh{tg}')
                    blk = ast[16 * bi:16 * bi + 16, :]
                    if r == 0:
                        nc.sync.dma_start(
                            qstf_d.ap()[16 * hl:16 * hl + 16, tg - 4], blk)
                    elif r == 16:
                        nc.sync.dma_start(agg_khold[tg][16 * hl:16 * hl + 16, :], blk)
                    else:
                        nc.sync.dma_start(agg_vhold[tg][16 * hl:16 * hl + 16, :], blk)

                # activate stage generators once a head group is complete
                if ct % 3 == 2:
                    kh, vh = qkv_holdq.pop(tgq)
                    pending.append(stage_a_gen(tgq, kh, vh, padded=True))
                    pending.append(stage_b_gen(tgq, padded=True))
                    tga = 4 + ct // 3
                    kh = agg_khold.pop(tga)
                    vh = agg_vhold.pop(tga)
                    qp = qbpfL if tga == 7 else qbpf
                    pending.append(stage_a_gen(tga, kh, vh, padded=False))
                    pending.append(stage_b_gen(tga, padded=False, qpool=qp))

            # ---------------- proj + BN, pipelined into the stage drain --------
            es_p1.close()
            wpp = ctx.enter_context(tc.tile_pool(name='wpp', bufs=1))
            wp = wpp.tile([128, 8, 512], BF16)
            nc.sync.dma_start(wp[:], wp_d.ap())
            bnbp = ctx.enter_context(tc.tile_pool(name='bnbp', bufs=1))
            bnb = bnbp.tile([128, 4], F32)
            nc.sync.dma_start(bnb[:], bnb_d.ap())

            atp = ctx.enter_context(tc.tile_pool(name='atp', bufs=2))
            ysp = ctx.enter_context(tc.tile_pool(name='ysp', bufs=3))
            at_tiles = {}

            def fetch_at(nt):
                # attn chunk nt is fully written once every tg emitted it
                while pending and min(sb_chunk.get(t, -1) for t in range(8)) < nt:
                    pump(1)
                at_tiles[nt] = atp.tile([128, 8, 512], BF16, tag='at',
                                        name=f'at{nt}')
                nc.sync.dma_start(at_tiles[nt][:],
                                  attn_d.ap()[:, :, 512 * nt:512 * (nt + 1)])

            fetch_at(0)
            for nt in range(NNT):
                if nt + 1 < NNT:
                    fetch_at(nt + 1)
                at = at_tiles.pop(nt)
                pump(2)
                for mt in range(4):
                    ps = psumB.tile([128, 512], F32, tag='psN', bufs=2)
                    for kt in range(8):
                        nc.tensor.matmul(ps[:], wp[:, kt, 128 * mt:128 * (mt + 1)],
                                         at[:, kt, :],
                                         start=(kt == 0), stop=(kt == 7))
                    ys = ysp.tile([128, 512], F32, tag='ys')
                    if mt % 2 == 0:
                        nc.vector.tensor_scalar_add(ys[:], ps[:], bnb[:, mt:mt + 1])
                    else:
                        nc.scalar.activation(ys[:], ps[:],
                                             mybir.ActivationFunctionType.Identity,
                                             bias=bnb[:, mt:mt + 1])
                    nc.sync.dma_start(
                        y_d.ap()[128 * mt:128 * (mt + 1), 512 * nt:512 * (nt + 1)],
                        ys[:])
            while pending:
                pump(1)

    nc.compile()
    return nc


def host_weights(w_qkv, w_dw, w_pw, w_proj, bn_gamma, bn_beta, bn_mean, bn_var):
    wq = w_qkv[:, :, 0, 0].astype(np.float32)       # [1536, 512]
    wdw = w_dw[:, 0].reshape(1536, 25).astype(np.float32)
    wpw = w_pw[:, :, 0, 0].astype(np.float32)       # [1536, 32]
    A = wdw.reshape(12, 4, 32, 25)
    Bm = wpw.reshape(12, 4, 32, 32)
    # W2c[ct, b, il, tap, ol] = dw[in-ch, tap] * pw[out-ch, in-ch]
    W2c = A[:, :, :, :, None] * Bm.transpose(0, 1, 3, 2)[:, :, :, None, :]
    lt = np.zeros((128, 12, 25, 128), np.float32)
    for b in range(4):
        lt[32 * b:32 * b + 32, :, :, 32 * b:32 * b + 32] = \
            W2c[:, b].transpose(1, 0, 2, 3)
    inv = bn_gamma / np.sqrt(bn_var + BN_EPS)
    wp_f = (w_proj[:, :, 0, 0] * inv[:, None]).T    # [1024, 512] lhsT
    bnb = (bn_beta - bn_mean * inv).astype(np.float32)

    wq_dev = np.ascontiguousarray(
        wq.T.reshape(NKT, 128, C3).transpose(1, 0, 2)).astype(ml_dtypes.bfloat16)
    lt_dev = lt.astype(ml_dtypes.bfloat16)
    wp_dev = np.ascontiguousarray(
        wp_f.reshape(8, 128, 512).transpose(1, 0, 2)).astype(ml_dtypes.bfloat16)
    bnb_dev = np.ascontiguousarray(bnb.reshape(4, 128).T).astype(np.float32)
    obd = np.zeros((128, 128), np.float32)
    for j in range(8):
        obd[16 * j:16 * j + 16, 16 * j:16 * j + 16] = 1.0
    obd_dev = obd.astype(ml_dtypes.bfloat16)
    return {'wq': wq_dev, 'lt': lt_dev, 'wp': wp_dev, 'bnb': bnb_dev, 'obd': obd_dev}


def kernel(x, w_qkv, w_dw, w_pw, w_proj, bn_gamma, bn_beta, bn_mean, bn_var):
    x = np.asarray(x, dtype=np.float32)
    wdev = host_weights(
        np.asarray(w_qkv, np.float32), np.asarray(w_dw, np.float32),
        np.asarray(w_pw, np.float32), np.asarray(w_proj, np.float32),
        np.asarray(bn_gamma, np.float32), np.asarray(bn_beta, np.float32),
        np.asarray(bn_mean, np.float32), np.asarray(bn_var, np.float32))

    if 'nc' not in _CACHED:
        _CACHED['nc'] = build_program()
    nc = _CACHED['nc']

    in_maps = []
    for b in range(B):
        xb = np.ascontiguousarray(
            x[b].reshape(NKT, 128, S).transpose(1, 0, 2)).astype(ml_dtypes.bfloat16)
        in_maps.append({'x16': xb, **wdev})
    _CACHED['in_maps'] = in_maps
    res = run_bass_kernel_spmd(nc, in_maps, list(range(B)))
    y = np.stack([res.results[b]['y_b'].reshape(512, H, W) for b in range(B)])
    return y.astype(np.float32)
